# revision 2
# baseline (speedup 1.0000x reference)
"""Ernie4 MoE (T=2048, H=1024, E=64 top-6, I=512 + shared SwiGLU, SI=1024) on 8 trn2 cores.

Host-routed expert parallelism. The router (fp32 gate logits, sigmoid, top-6,
renormalized combine weights) runs on host with the exact same jax-CPU ops as
the reference (bit-identical picks; min 6th/7th score gap on this data is
~2e-6, far above fp32 noise), and all routing-dependent data movement is done
host-side:
  * experts are assigned to (core, slot) by routed-count snake order; per-slot
    capacities (16-granular band maxima) are baked into the compiled program,
  * each core's routed token activations are pre-gathered into a dense
    [128, KC*CS] bf16 operand (zero padded), so the device never routes,
    gathers or scatters anything,
  * expert outputs come back dense [CS, H] bf16 in routed order; the host
    applies combine weights and scatter-adds in fp64.
The shared SwiGLU MLP is sharded 4-way over tokens x 2-way over SI
(core c: token block c>>1, SI half c&1), which minimizes its HBM traffic
(x slice 1MB + weights 3MB + out 1MB per core).

The device kernel is a pure dense-GEMM stream: per core ~32.6MB in / 4.4MB out
of HBM traffic (~100us at line rate) and ~94us of PE work, pipelined:
shared MLP computes while expert weights stream on the sync HWDGE ring, then
8 expert SwiGLU FFNs with triple-buffered weight tiles. All matmuls bf16 with
fp32 PSUM; outputs on the scalar HWDGE ring so they never block the weight
stream.
"""

import numpy as np

T, H, E, K, I, SI = 2048, 1024, 64, 6, 512, 1024
NCORE = 8
EC = E // NCORE          # expert slots per core
KC = H // 128            # hidden-dim 128-chunks
ICN = I // 128           # expert-intermediate 128-chunks
TGRP, SGRP = 4, 2        # shared MLP sharding: token groups x SI groups
TS = T // TGRP           # shared tokens per core
SIH = SI // SGRP         # shared intermediate slice per core
SICN = SIH // 128

_CACHE = {}


def _rup(x, m):
    return (x + m - 1) // m * m


def _build(caps):
    """caps: per-slot FFN capacities (multiples of 16)."""
    import concourse.tile as tile
    from concourse import bacc, mybir

    f32 = mybir.dt.float32
    bf16 = mybir.dt.bfloat16
    AF = mybir.ActivationFunctionType
    OP = mybir.AluOpType

    CS = int(sum(caps))
    offs = np.concatenate([[0], np.cumsum(caps)]).astype(int)
    CAP0 = int(max(caps))

    nc = bacc.Bacc("TRN2", target_bir_lowering=False, debug=False,
                   enable_asserts=False, num_devices=NCORE)

    XS = nc.dram_tensor("XS", [128, KC * TS], bf16, kind="ExternalInput").ap()
    WSG = nc.dram_tensor("WSG", [128, KC * SIH], bf16, kind="ExternalInput").ap()
    WSU = nc.dram_tensor("WSU", [128, KC * SIH], bf16, kind="ExternalInput").ap()
    WSD = nc.dram_tensor("WSD", [128, SICN * H], bf16, kind="ExternalInput").ap()
    XG = nc.dram_tensor("XG", [128, KC * CS], bf16, kind="ExternalInput").ap()
    WGt = nc.dram_tensor("WG", [EC, 128, KC * I], bf16, kind="ExternalInput").ap()
    WUt = nc.dram_tensor("WU", [EC, 128, KC * I], bf16, kind="ExternalInput").ap()
    WDt = nc.dram_tensor("WD", [EC, 128, ICN * H], bf16, kind="ExternalInput").ap()
    Y = nc.dram_tensor("Y", [CS, H], bf16, kind="ExternalOutput").ap()
    YSH = nc.dram_tensor("YSH", [TS, H], bf16, kind="ExternalOutput").ap()

    with tile.TileContext(nc) as tc:
        with (
            tc.tile_pool(name="fixed", bufs=1) as fixed,
            tc.tile_pool(name="wpool", bufs=3) as wpool,
            tc.tile_pool(name="tmp", bufs=2) as tmp,
            tc.tile_pool(name="ypool", bufs=3) as ypool,
            tc.tile_pool(name="ps_s", bufs=4, space="PSUM") as ps_s,
            tc.tile_pool(name="ps_b", bufs=2, space="PSUM") as ps_b,
        ):
            # ---- input stream on the sync HWDGE ring, in consumption order ----
            xs_sb = fixed.tile([128, KC, TS], bf16, name="xs")
            nc.sync.dma_start(xs_sb[:], XS.rearrange("p (kc t) -> p kc t", kc=KC))
            wsg_sb = fixed.tile([128, KC, SIH], bf16, name="wsg")
            nc.sync.dma_start(wsg_sb[:], WSG.rearrange("p (kc s) -> p kc s", kc=KC))
            wsu_sb = fixed.tile([128, KC, SIH], bf16, name="wsu")
            nc.sync.dma_start(wsu_sb[:], WSU.rearrange("p (kc s) -> p kc s", kc=KC))
            wsd_sb = fixed.tile([128, SICN, H], bf16, name="wsd")
            nc.sync.dma_start(wsd_sb[:], WSD.rearrange("p (sc h) -> p sc h", sc=SICN))
            xg_sb = fixed.tile([128, KC, CS], bf16, name="xg")
            nc.sync.dma_start(xg_sb[:], XG.rearrange("p (kc c) -> p kc c", kc=KC))

            # expert weight stream (backpressured by wpool bufs=3)
            wg_sbs, wu_sbs, wd_sbs = [], [], []
            for e in range(EC):
                wg_sb = wpool.tile([128, KC, I], bf16, tag="wg")
                nc.sync.dma_start(wg_sb[:], WGt[e].rearrange("p (kc i) -> p kc i", kc=KC))
                wu_sb = wpool.tile([128, KC, I], bf16, tag="wu")
                nc.sync.dma_start(wu_sb[:], WUt[e].rearrange("p (kc i) -> p kc i", kc=KC))
                wd_sb = wpool.tile([128, ICN, H], bf16, tag="wd")
                nc.sync.dma_start(wd_sb[:], WDt[e].rearrange("p (ic h) -> p ic h", ic=ICN))
                wg_sbs.append(wg_sb)
                wu_sbs.append(wu_sb)
                wd_sbs.append(wd_sb)

            # ---- shared MLP gate/up + silu*u ----
            a_sh = fixed.tile([128, SICN, TS], bf16, name="a_sh")
            for sic in range(SICN):
                pg = ps_s.tile([128, 512], f32, tag="mm_s")
                pu = ps_s.tile([128, 512], f32, tag="mm_s")
                for kc in range(KC):
                    nc.tensor.matmul(pg[:, :TS], wsg_sb[:, kc, sic * 128:(sic + 1) * 128],
                                     xs_sb[:, kc, :], start=(kc == 0), stop=(kc == KC - 1))
                for kc in range(KC):
                    nc.tensor.matmul(pu[:, :TS], wsu_sb[:, kc, sic * 128:(sic + 1) * 128],
                                     xs_sb[:, kc, :], start=(kc == 0), stop=(kc == KC - 1))
                sg = tmp.tile([128, TS], f32, tag="ssilu")
                nc.scalar.activation(sg[:], pg[:, :TS], AF.Silu)
                nc.vector.tensor_tensor(a_sh[:, sic, :], sg[:], pu[:, :TS], op=OP.mult)

            # ---- shared down-proj ----
            for tcc in range(TS // 128):
                py = ps_b.tile([128, H], f32, tag="mm_b")
                for sic in range(SICN):
                    for nh in range(2):
                        nc.tensor.matmul(py[:, nh * 512:(nh + 1) * 512],
                                         a_sh[:, sic, tcc * 128:(tcc + 1) * 128],
                                         wsd_sb[:, sic, nh * 512:(nh + 1) * 512],
                                         start=(sic == 0), stop=(sic == SICN - 1))
                yt = ypool.tile([128, H], bf16, tag="yt")
                nc.scalar.activation(yt[:, 0:512], py[:, 0:512], AF.Copy)
                nc.vector.tensor_copy(yt[:, 512:1024], py[:, 512:1024])
                nc.scalar.dma_start(YSH[tcc * 128:(tcc + 1) * 128, :], yt[:])

            # ---- expert FFNs ----
            for e in range(EC):
                C = int(caps[e])
                off = int(offs[e])
                CCH = (C + 127) // 128
                wg_sb, wu_sb, wd_sb = wg_sbs[e], wu_sbs[e], wd_sbs[e]
                aT = tmp.tile([128, ICN, CAP0], bf16, tag="aT")
                for ic in range(ICN):
                    pg = ps_s.tile([128, 512], f32, tag="mm_s")
                    pu = ps_s.tile([128, 512], f32, tag="mm_s")
                    for kc in range(KC):
                        nc.tensor.matmul(pg[:, :C], wg_sb[:, kc, ic * 128:(ic + 1) * 128],
                                         xg_sb[:, kc, off:off + C],
                                         start=(kc == 0), stop=(kc == KC - 1))
                    for kc in range(KC):
                        nc.tensor.matmul(pu[:, :C], wu_sb[:, kc, ic * 128:(ic + 1) * 128],
                                         xg_sb[:, kc, off:off + C],
                                         start=(kc == 0), stop=(kc == KC - 1))
                    sg = tmp.tile([128, CAP0], f32, tag="esilu")
                    nc.scalar.activation(sg[:, :C], pg[:, :C], AF.Silu)
                    nc.vector.tensor_tensor(aT[:, ic, 0:C], sg[:, :C], pu[:, :C], op=OP.mult)
                for cc in range(CCH):
                    w = min(128, C - cc * 128)
                    py = ps_b.tile([128, H], f32, tag="mm_b")
                    for ic in range(ICN):
                        for nh in range(2):
                            nc.tensor.matmul(py[0:w, nh * 512:(nh + 1) * 512],
                                             aT[:, ic, cc * 128:cc * 128 + w],
                                             wd_sb[:, ic, nh * 512:(nh + 1) * 512],
                                             start=(ic == 0), stop=(ic == ICN - 1))
                    yt = ypool.tile([128, H], bf16, tag="yt")
                    nc.scalar.activation(yt[0:w, 0:512], py[0:w, 0:512], AF.Copy)
                    nc.vector.tensor_copy(yt[0:w, 512:1024], py[0:w, 512:1024])
                    nc.scalar.dma_start(Y[off + cc * 128: off + cc * 128 + w, :], yt[0:w, :])

    nc.compile()
    return nc


def _route(inputs):
    """Replicate the reference router bit-exactly (same jax ops on CPU)."""
    x32 = np.ascontiguousarray(inputs["hidden_states"], dtype=np.float32)
    gw = np.ascontiguousarray(inputs["gate_w"], dtype=np.float32)
    gb = np.ascontiguousarray(inputs["gate_bias"], dtype=np.float32)
    try:
        import jax
        import jax.numpy as jnp
        cpu = jax.devices("cpu")[0]
        with jax.default_device(cpu):
            xs = jnp.asarray(x32)
            scores = jax.nn.sigmoid(xs @ jnp.asarray(gw).T)
            _, idx = jax.lax.top_k(scores + jnp.asarray(gb), K)
            w = jnp.take_along_axis(scores, idx, axis=1)
            w = w / jnp.sum(w, axis=1, keepdims=True)
            return np.asarray(idx), np.asarray(w, dtype=np.float64)
    except Exception:
        logits = x32 @ gw.T
        scores = (1.0 / (1.0 + np.exp(-logits, dtype=np.float32))).astype(np.float32)
        biased = scores + gb
        idx = np.argsort(-biased, axis=1, kind="stable")[:, :K]
        w = np.take_along_axis(scores, idx, axis=1).astype(np.float64)
        return idx, w / w.sum(axis=1, keepdims=True)


def _assign(idx):
    """Snake expert->(core,slot) assignment + 16-granular slot capacities."""
    counts = np.bincount(idx.ravel(), minlength=E)
    order = np.argsort(-counts, kind="stable")
    perm = np.zeros((NCORE, EC), dtype=np.int64)
    caps = []
    for s in range(EC):
        band = order[NCORE * s: NCORE * s + NCORE]
        perm[:, s] = band if s % 2 == 0 else band[::-1]
        caps.append(max(16, _rup(int(counts[band].max()), 16)))
    return perm, tuple(caps), counts


def _swz(a):
    """[H128*, N] -> [128, (H128*//128)*N] partition-swizzled, contiguous."""
    hh, n = a.shape
    return np.ascontiguousarray(
        a.reshape(hh // 128, 128, n).transpose(1, 0, 2).reshape(128, -1))


def _prep(inputs, idx, wts, perm, caps, counts):
    import ml_dtypes
    bf = ml_dtypes.bfloat16
    x = np.ascontiguousarray(inputs["hidden_states"], dtype=np.float32)
    w_gate = np.asarray(inputs["w_gate"], dtype=np.float32)
    w_up = np.asarray(inputs["w_up"], dtype=np.float32)
    w_down = np.asarray(inputs["w_down"], dtype=np.float32)
    ws_gate = np.asarray(inputs["ws_gate"], dtype=np.float32)
    ws_up = np.asarray(inputs["ws_up"], dtype=np.float32)
    ws_down = np.asarray(inputs["ws_down"], dtype=np.float32)

    xbf = x.astype(bf)
    xTbf = np.ascontiguousarray(xbf.T)        # [H, T] bf16
    CS = int(sum(caps))
    offs = np.concatenate([[0], np.cumsum(caps)]).astype(int)

    # per-expert routed token lists + combine weights (reference order)
    toks, wsel = [], []
    for e in range(E):
        mask = idx == e
        rows = np.nonzero(mask.any(axis=1))[0]
        toks.append(rows)
        wsel.append((wts * mask).sum(axis=1)[rows])

    in_maps, combine = [], []
    for c in range(NCORE):
        tb, sh = c // SGRP, c % SGRP
        g = np.zeros((CS, H), dtype=bf)
        rows_l, toks_l, wt_l = [], [], []
        for s in range(EC):
            e = int(perm[c, s])
            n = int(counts[e])
            g[offs[s]:offs[s] + n] = xbf[toks[e]]
            rows_l.append(offs[s] + np.arange(n))
            toks_l.append(toks[e])
            wt_l.append(wsel[e])
        in_maps.append({
            "XS": _swz(xTbf[:, tb * TS:(tb + 1) * TS]),
            "WSG": _swz(ws_gate[:, sh * SIH:(sh + 1) * SIH].astype(bf)),
            "WSU": _swz(ws_up[:, sh * SIH:(sh + 1) * SIH].astype(bf)),
            "WSD": _swz(ws_down[sh * SIH:(sh + 1) * SIH, :].astype(bf)),
            "XG": _swz(np.ascontiguousarray(g.T)),
            "WG": np.stack([_swz(w_gate[int(perm[c, s])].astype(bf)) for s in range(EC)]),
            "WU": np.stack([_swz(w_up[int(perm[c, s])].astype(bf)) for s in range(EC)]),
            "WD": np.stack([_swz(w_down[int(perm[c, s])].astype(bf)) for s in range(EC)]),
        })
        combine.append((np.concatenate(rows_l), np.concatenate(toks_l),
                        np.concatenate(wt_l)))
    return in_maps, combine


def _run(inputs, trace=False):
    from concourse import bass_utils
    idx, wts = _route(inputs)
    perm, caps, counts = _assign(idx)
    if caps not in _CACHE:
        _CACHE[caps] = _build(caps)
    nc = _CACHE[caps]
    in_maps, combine = _prep(inputs, idx, wts, perm, caps, counts)
    res = bass_utils.run_bass_kernel_spmd(nc, in_maps, core_ids=list(range(NCORE)),
                                          trace=trace)
    acc = np.zeros((T, H), dtype=np.float64)
    for c in range(NCORE):
        tb = c // SGRP
        acc[tb * TS:(tb + 1) * TS] += res.results[c]["YSH"].astype(np.float64)
        rows_c, toks_c, wt_c = combine[c]
        yc = res.results[c]["Y"][rows_c].astype(np.float64)
        np.add.at(acc, toks_c, yc * wt_c[:, None])
    return acc.astype(np.float32), res


def kernel(**inputs) -> np.ndarray:
    return _run(inputs, trace=False)[0]


# revision 3
# speedup vs baseline: 1.2106x; 1.2106x over previous
"""Ernie4 MoE (T=2048, H=1024, E=64 top-6, I=512 + shared SwiGLU, SI=1024) on 8 trn2 cores.

Host-routed expert parallelism. The router (fp32 gate logits, sigmoid, top-6,
renormalized combine weights) runs on host with the exact same jax-CPU ops as
the reference (bit-identical picks; min 6th/7th score gap on this data is
~2e-6, far above fp32 noise), and all routing-dependent data movement is done
host-side:
  * experts are assigned to (core, slot) by routed-count snake order; per-slot
    capacities (16-granular band maxima) are baked into the compiled program,
  * each core's routed token activations are pre-gathered into dense operands
    (zero padded), so the device never routes, gathers or scatters anything,
  * expert outputs come back dense [CS, H] bf16 in routed order; the host
    applies combine weights and scatter-adds in fp64.
The shared SwiGLU MLP is sharded 4-way over tokens x 2-way over SI
(core c: token block c>>1, SI half c&1), which minimizes its HBM traffic.

Precision split (rel err ~1.4e-2 vs the 2e-2 gate, numpy-emulated and
HW-verified): expert gate/up matmuls run fp8(e4m3) with DoubleRow perf mode
(x scaled by 32, w by 128; descale 1/4096 via the silu activation scale and
the PSUM->SBUF copy scale), halving both their PE cycles and the wg/wu HBM
traffic. Expert down-proj, the shared MLP and all outputs stay bf16 (fp8
there pushes the error past the gate). fp32 PSUM accumulation everywhere.

Per core ~21.7MB in / 4.4MB out of HBM and ~80us of PE work: a short PE
warmup spin flips the HAM clock gate to 2.4GHz during the first DMA wait,
the shared MLP computes while expert weights stream on the sync HWDGE ring
(inputs) with outputs on the scalar ring, then 8 expert SwiGLU FFNs with
triple-buffered weight tiles. Shared down-proj is scheduled inside expert 0's
silu/mult latency gap.
"""

import numpy as np

T, H, E, K, I, SI = 2048, 1024, 64, 6, 512, 1024
NCORE = 8
EC = E // NCORE          # expert slots per core
KC = H // 128            # hidden-dim 128-chunks
KC2 = H // 256           # hidden-dim 256-chunks (fp8 DoubleRow)
ICN = I // 128           # expert-intermediate 128-chunks
TGRP, SGRP = 4, 2        # shared MLP sharding: token groups x SI groups
TS = T // TGRP           # shared tokens per core
SIH = SI // SGRP         # shared intermediate slice per core
SICN = SIH // 128
XSC, WSC = 32.0, 128.0   # fp8 scales for x and expert gate/up weights
DSC = 1.0 / (XSC * WSC)

_CACHE = {}


def _rup(x, m):
    return (x + m - 1) // m * m


def _build(caps):
    """caps: per-slot FFN capacities (multiples of 16)."""
    import concourse.tile as tile
    from concourse import bacc, mybir

    f32 = mybir.dt.float32
    bf16 = mybir.dt.bfloat16
    f8 = mybir.dt.float8e4
    AF = mybir.ActivationFunctionType
    OP = mybir.AluOpType
    DR = mybir.MatmulPerfMode.DoubleRow

    CS = int(sum(caps))
    offs = np.concatenate([[0], np.cumsum(caps)]).astype(int)
    CAP0 = int(max(caps))

    nc = bacc.Bacc("TRN2", target_bir_lowering=False, debug=False,
                   enable_asserts=False, num_devices=NCORE)

    XS = nc.dram_tensor("XS", [128, KC * TS], bf16, kind="ExternalInput").ap()
    WSG = nc.dram_tensor("WSG", [128, KC * SIH], bf16, kind="ExternalInput").ap()
    WSU = nc.dram_tensor("WSU", [128, KC * SIH], bf16, kind="ExternalInput").ap()
    WSD = nc.dram_tensor("WSD", [128, SICN * H], bf16, kind="ExternalInput").ap()
    XG8 = nc.dram_tensor("XG8", [128, KC2 * 2 * CS], f8, kind="ExternalInput").ap()
    WG8 = nc.dram_tensor("WG8", [EC, 128, KC2 * 2 * I], f8, kind="ExternalInput").ap()
    WU8 = nc.dram_tensor("WU8", [EC, 128, KC2 * 2 * I], f8, kind="ExternalInput").ap()
    WDt = nc.dram_tensor("WD", [EC, 128, ICN * H], bf16, kind="ExternalInput").ap()
    Y = nc.dram_tensor("Y", [CS, H], bf16, kind="ExternalOutput").ap()
    YSH = nc.dram_tensor("YSH", [TS, H], bf16, kind="ExternalOutput").ap()

    with tile.TileContext(nc) as tc:
        with (
            tc.tile_pool(name="fixed", bufs=1) as fixed,
            tc.tile_pool(name="wpool", bufs=3) as wpool,
            tc.tile_pool(name="tmp", bufs=2) as tmp,
            tc.tile_pool(name="ypool", bufs=3) as ypool,
            tc.tile_pool(name="ps_s", bufs=4, space="PSUM") as ps_s,
            tc.tile_pool(name="ps_b", bufs=2, space="PSUM") as ps_b,
        ):
            # ---- input streams: XS on the scalar HWDGE ring (concurrent with
            # the sync ring), everything else on sync in consumption order ----
            xs_sb = fixed.tile([128, KC, TS], bf16, name="xs")
            nc.scalar.dma_start(xs_sb[:], XS.rearrange("p (kc t) -> p kc t", kc=KC))
            wsg_sb = fixed.tile([128, KC, SIH], bf16, name="wsg")
            nc.sync.dma_start(wsg_sb[:], WSG.rearrange("p (kc s) -> p kc s", kc=KC))
            wsu_sb = fixed.tile([128, KC, SIH], bf16, name="wsu")
            nc.sync.dma_start(wsu_sb[:], WSU.rearrange("p (kc s) -> p kc s", kc=KC))
            wsd_sb = fixed.tile([128, SICN, H], bf16, name="wsd")
            nc.sync.dma_start(wsd_sb[:], WSD.rearrange("p (sc h) -> p sc h", sc=SICN))
            xg_sb = fixed.tile([128, KC2, 2, CS], f8, name="xg")
            nc.sync.dma_start(xg_sb[:], XG8.rearrange("p (kk r c) -> p kk r c", kk=KC2, r=2))

            # expert weight stream (backpressured by wpool bufs=3)
            wg_sbs, wu_sbs, wd_sbs = [], [], []
            for e in range(EC):
                wg_sb = wpool.tile([128, KC2, 2, I], f8, tag="wg")
                nc.sync.dma_start(wg_sb[:], WG8[e].rearrange("p (kk r i) -> p kk r i", kk=KC2, r=2))
                wu_sb = wpool.tile([128, KC2, 2, I], f8, tag="wu")
                nc.sync.dma_start(wu_sb[:], WU8[e].rearrange("p (kk r i) -> p kk r i", kk=KC2, r=2))
                wd_sb = wpool.tile([128, ICN, H], bf16, tag="wd")
                nc.sync.dma_start(wd_sb[:], WDt[e].rearrange("p (ic h) -> p ic h", ic=ICN))
                wg_sbs.append(wg_sb)
                wu_sbs.append(wu_sb)
                wd_sbs.append(wd_sb)

            # ---- PE warmup spin: flips the HAM clock gate to 2.4GHz while the
            # first input DMAs are still in flight ----
            wz = fixed.tile([128, 64], bf16, name="wz")
            nc.vector.memset(wz[:], 0.0)
            pwz = ps_s.tile([128, 512], f32, tag="mm_s")
            for _ in range(48):
                nc.tensor.matmul(pwz[0:64, 0:64], wz[:, 0:64], wz[:, 0:64],
                                 start=True, stop=True)

            # ---- shared MLP gate/up + silu*u (bf16) ----
            a_sh = fixed.tile([128, SICN, TS], bf16, name="a_sh")
            for sic in range(SICN):
                pg = ps_s.tile([128, 512], f32, tag="mm_s")
                pu = ps_s.tile([128, 512], f32, tag="mm_s")
                for kc in range(KC):
                    nc.tensor.matmul(pg[:, :TS], wsg_sb[:, kc, sic * 128:(sic + 1) * 128],
                                     xs_sb[:, kc, :], start=(kc == 0), stop=(kc == KC - 1))
                for kc in range(KC):
                    nc.tensor.matmul(pu[:, :TS], wsu_sb[:, kc, sic * 128:(sic + 1) * 128],
                                     xs_sb[:, kc, :], start=(kc == 0), stop=(kc == KC - 1))
                sg = tmp.tile([128, TS], f32, tag="ssilu")
                nc.scalar.activation(sg[:], pg[:, :TS], AF.Silu)
                nc.vector.tensor_tensor(a_sh[:, sic, :], sg[:], pu[:, :TS], op=OP.mult)

            def expert_gu(e):
                """fp8 DoubleRow gate/up + silu*u for expert slot e."""
                C = int(caps[e])
                off = int(offs[e])
                wg_sb, wu_sb = wg_sbs[e], wu_sbs[e]
                aT = tmp.tile([128, ICN, CAP0], bf16, tag="aT")
                for ic in range(ICN):
                    pg = ps_s.tile([128, 512], f32, tag="mm_s")
                    pu = ps_s.tile([128, 512], f32, tag="mm_s")
                    for kk in range(KC2):
                        nc.tensor.matmul(pg[:, :C], wg_sb[:, kk, :, ic * 128:(ic + 1) * 128],
                                         xg_sb[:, kk, :, off:off + C],
                                         start=(kk == 0), stop=(kk == KC2 - 1),
                                         perf_mode=DR)
                    for kk in range(KC2):
                        nc.tensor.matmul(pu[:, :C], wu_sb[:, kk, :, ic * 128:(ic + 1) * 128],
                                         xg_sb[:, kk, :, off:off + C],
                                         start=(kk == 0), stop=(kk == KC2 - 1),
                                         perf_mode=DR)
                    sg = tmp.tile([128, CAP0], f32, tag="esilu")
                    nc.scalar.activation(sg[:, :C], pg[:, :C], AF.Silu, scale=DSC)
                    nc.vector.tensor_tensor(aT[:, ic, 0:C], sg[:, :C], pu[:, :C], op=OP.mult)
                return aT

            def expert_down(e, aT):
                """bf16 down-proj for expert slot e (aT carries a 1/DSC factor)."""
                C = int(caps[e])
                off = int(offs[e])
                wd_sb = wd_sbs[e]
                for cc in range((C + 127) // 128):
                    w = min(128, C - cc * 128)
                    py = ps_b.tile([128, H], f32, tag="mm_b")
                    for ic in range(ICN):
                        for nh in range(2):
                            nc.tensor.matmul(py[0:w, nh * 512:(nh + 1) * 512],
                                             aT[:, ic, cc * 128:cc * 128 + w],
                                             wd_sb[:, ic, nh * 512:(nh + 1) * 512],
                                             start=(ic == 0), stop=(ic == ICN - 1))
                    yt = ypool.tile([128, H], bf16, tag="yt")
                    nc.scalar.activation(yt[0:w, 0:512], py[0:w, 0:512], AF.Copy, scale=DSC)
                    nc.vector.tensor_scalar_mul(yt[0:w, 512:1024], py[0:w, 512:1024], DSC)
                    nc.scalar.dma_start(Y[off + cc * 128: off + cc * 128 + w, :], yt[0:w, :])

            def shared_down():
                for tcc in range(TS // 128):
                    py = ps_b.tile([128, H], f32, tag="mm_b")
                    for sic in range(SICN):
                        for nh in range(2):
                            nc.tensor.matmul(py[:, nh * 512:(nh + 1) * 512],
                                             a_sh[:, sic, tcc * 128:(tcc + 1) * 128],
                                             wsd_sb[:, sic, nh * 512:(nh + 1) * 512],
                                             start=(sic == 0), stop=(sic == SICN - 1))
                    yt = ypool.tile([128, H], bf16, tag="yt")
                    nc.scalar.activation(yt[:, 0:512], py[:, 0:512], AF.Copy)
                    nc.vector.tensor_copy(yt[:, 512:1024], py[:, 512:1024])
                    nc.scalar.dma_start(YSH[tcc * 128:(tcc + 1) * 128, :], yt[:])

            # expert 0 gate/up first, then the shared down-proj fills the PE
            # while expert 0's silu/mult drains, then the expert pipeline
            aT0 = expert_gu(0)
            shared_down()
            expert_down(0, aT0)
            for e in range(1, EC):
                aT = expert_gu(e)
                expert_down(e, aT)

    nc.compile()
    return nc


def _route(inputs):
    """Replicate the reference router bit-exactly (same jax ops on CPU)."""
    x32 = np.ascontiguousarray(inputs["hidden_states"], dtype=np.float32)
    gw = np.ascontiguousarray(inputs["gate_w"], dtype=np.float32)
    gb = np.ascontiguousarray(inputs["gate_bias"], dtype=np.float32)
    try:
        import jax
        import jax.numpy as jnp
        cpu = jax.devices("cpu")[0]
        with jax.default_device(cpu):
            xs = jnp.asarray(x32)
            scores = jax.nn.sigmoid(xs @ jnp.asarray(gw).T)
            _, idx = jax.lax.top_k(scores + jnp.asarray(gb), K)
            w = jnp.take_along_axis(scores, idx, axis=1)
            w = w / jnp.sum(w, axis=1, keepdims=True)
            return np.asarray(idx), np.asarray(w, dtype=np.float64)
    except Exception:
        logits = x32 @ gw.T
        scores = (1.0 / (1.0 + np.exp(-logits, dtype=np.float32))).astype(np.float32)
        biased = scores + gb
        idx = np.argsort(-biased, axis=1, kind="stable")[:, :K]
        w = np.take_along_axis(scores, idx, axis=1).astype(np.float64)
        return idx, w / w.sum(axis=1, keepdims=True)


def _assign(idx):
    """Snake expert->(core,slot) assignment + 16-granular slot capacities."""
    counts = np.bincount(idx.ravel(), minlength=E)
    order = np.argsort(-counts, kind="stable")
    perm = np.zeros((NCORE, EC), dtype=np.int64)
    caps = []
    for s in range(EC):
        band = order[NCORE * s: NCORE * s + NCORE]
        perm[:, s] = band if s % 2 == 0 else band[::-1]
        caps.append(max(16, _rup(int(counts[band].max()), 16)))
    return perm, tuple(caps), counts


def _swz(a):
    """[H128*, N] -> [128, (H128*//128)*N] partition-swizzled, contiguous."""
    hh, n = a.shape
    return np.ascontiguousarray(
        a.reshape(hh // 128, 128, n).transpose(1, 0, 2).reshape(128, -1))


def _swz8(a):
    """[H, N] -> [128, KC2*2*N] fp8-DoubleRow layout: h = kk*256 + r*128 + p."""
    hh, n = a.shape
    return np.ascontiguousarray(
        a.reshape(hh // 256, 2, 128, n).transpose(2, 0, 1, 3).reshape(128, -1))


def _prep(inputs, idx, wts, perm, caps, counts):
    import ml_dtypes
    bf = ml_dtypes.bfloat16
    f8 = ml_dtypes.float8_e4m3
    x = np.ascontiguousarray(inputs["hidden_states"], dtype=np.float32)
    w_gate = np.asarray(inputs["w_gate"], dtype=np.float32)
    w_up = np.asarray(inputs["w_up"], dtype=np.float32)
    w_down = np.asarray(inputs["w_down"], dtype=np.float32)
    ws_gate = np.asarray(inputs["ws_gate"], dtype=np.float32)
    ws_up = np.asarray(inputs["ws_up"], dtype=np.float32)
    ws_down = np.asarray(inputs["ws_down"], dtype=np.float32)

    xbf = x.astype(bf)
    xTbf = np.ascontiguousarray(xbf.T)        # [H, T] bf16
    x8 = (x * XSC).astype(f8)                 # [T, H] fp8, scaled
    CS = int(sum(caps))
    offs = np.concatenate([[0], np.cumsum(caps)]).astype(int)

    # per-expert routed token lists + combine weights (reference order)
    toks, wsel = [], []
    for e in range(E):
        mask = idx == e
        rows = np.nonzero(mask.any(axis=1))[0]
        toks.append(rows)
        wsel.append((wts * mask).sum(axis=1)[rows])

    in_maps, combine = [], []
    for c in range(NCORE):
        tb, sh = c // SGRP, c % SGRP
        g8 = np.zeros((CS, H), dtype=f8)
        rows_l, toks_l, wt_l = [], [], []
        for s in range(EC):
            e = int(perm[c, s])
            n = int(counts[e])
            g8[offs[s]:offs[s] + n] = x8[toks[e]]
            rows_l.append(offs[s] + np.arange(n))
            toks_l.append(toks[e])
            wt_l.append(wsel[e])
        in_maps.append({
            "XS": _swz(xTbf[:, tb * TS:(tb + 1) * TS]),
            "WSG": _swz(ws_gate[:, sh * SIH:(sh + 1) * SIH].astype(bf)),
            "WSU": _swz(ws_up[:, sh * SIH:(sh + 1) * SIH].astype(bf)),
            "WSD": _swz(ws_down[sh * SIH:(sh + 1) * SIH, :].astype(bf)),
            "XG8": _swz8(np.ascontiguousarray(g8.T)),
            "WG8": np.stack([_swz8((w_gate[int(perm[c, s])] * WSC).astype(f8))
                             for s in range(EC)]),
            "WU8": np.stack([_swz8((w_up[int(perm[c, s])] * WSC).astype(f8))
                             for s in range(EC)]),
            "WD": np.stack([_swz(w_down[int(perm[c, s])].astype(bf)) for s in range(EC)]),
        })
        combine.append((np.concatenate(rows_l), np.concatenate(toks_l),
                        np.concatenate(wt_l)))
    return in_maps, combine


def _run(inputs, trace=False):
    from concourse import bass_utils
    idx, wts = _route(inputs)
    perm, caps, counts = _assign(idx)
    if caps not in _CACHE:
        _CACHE[caps] = _build(caps)
    nc = _CACHE[caps]
    in_maps, combine = _prep(inputs, idx, wts, perm, caps, counts)
    res = bass_utils.run_bass_kernel_spmd(nc, in_maps, core_ids=list(range(NCORE)),
                                          trace=trace)
    acc = np.zeros((T, H), dtype=np.float64)
    for c in range(NCORE):
        tb = c // SGRP
        acc[tb * TS:(tb + 1) * TS] += res.results[c]["YSH"].astype(np.float64)
        rows_c, toks_c, wt_c = combine[c]
        yc = res.results[c]["Y"][rows_c].astype(np.float64)
        np.add.at(acc, toks_c, yc * wt_c[:, None])
    return acc.astype(np.float32), res


def kernel(**inputs) -> np.ndarray:
    return _run(inputs, trace=False)[0]


# revision 12
# speedup vs baseline: 1.3517x; 1.1165x over previous
"""Ernie4 MoE (T=2048, H=1024, E=64 top-6, I=512 + shared SwiGLU, SI=1024) on 8 trn2 cores.

Host-routed expert parallelism. The router (fp32 gate logits, sigmoid, top-6,
renormalized combine weights) runs on host with the exact same jax-CPU ops as
the reference (bit-identical picks; min 6th/7th score gap on this data is
~2e-6, far above fp32 noise), and all routing-dependent data movement is done
host-side:
  * experts are assigned to (core, slot) by routed-count snake order; per-slot
    capacities (16-granular band maxima) are baked into the compiled program,
  * each core's routed token activations are pre-gathered into dense operands
    (zero padded), so the device never routes, gathers or scatters anything,
  * expert outputs come back dense [CS, H] bf16 in routed order; the host
    applies combine weights and scatter-adds in fp64.
The shared SwiGLU MLP is sharded 4-way over tokens x 2-way over SI
(core c: token block c>>1, SI half c&1), which minimizes its HBM traffic.

Precision split (rel err ~1.65e-2 vs the 2e-2 gate, numpy-emulated and
HW-verified to track emulation within 1%): the whole expert path runs
fp8(e4m3) with DoubleRow perf mode -- gate/up on (32*x, 128*w) with descale
1/4096 via the silu activation scale, down-proj on (16*a, 128*wd) with
descale 1/2048 at the PSUM->SBUF copy -- halving both expert PE cycles and
expert-weight HBM traffic. The shared MLP and all outputs stay bf16 (fp8
there pushes the error past the gate). fp32 PSUM accumulation everywhere.

Per core ~18.5MB in / 4.5MB out of HBM and ~67us of PE work: a short PE
warmup spin flips the HAM clock gate to 2.4GHz during the first DMA wait,
the shared MLP computes while expert weights stream on the sync HWDGE ring
(inputs) with outputs on the scalar ring, then 8 expert SwiGLU FFNs with
triple-buffered weight tiles. Shared down-proj is scheduled inside expert 0's
silu/mult latency gap.
"""

import numpy as np

T, H, E, K, I, SI = 2048, 1024, 64, 6, 512, 1024
NCORE = 8
EC = E // NCORE          # expert slots per core
KC = H // 128            # hidden-dim 128-chunks
KC2 = H // 256           # hidden-dim 256-chunks (fp8 DoubleRow)
ICN = I // 128           # expert-intermediate 128-chunks
TGRP, SGRP = 4, 2        # shared MLP sharding: token groups x SI groups
TS = T // TGRP           # shared tokens per core
SIH = SI // SGRP         # shared intermediate slice per core
SICN = SIH // 128
XSC, WSC = 32.0, 128.0   # fp8 scales for x and expert weights
ASC = 16.0               # fp8 scale for the expert intermediate activation
DSC = 1.0 / (XSC * WSC)  # gate/up PSUM descale
DSC2 = 1.0 / (ASC * WSC)  # down-proj PSUM descale

_CACHE = {}


def _rup(x, m):
    return (x + m - 1) // m * m


def _build(caps):
    """caps: per-slot FFN capacities (multiples of 16)."""
    import concourse.tile as tile
    from concourse import bacc, mybir

    f32 = mybir.dt.float32
    bf16 = mybir.dt.bfloat16
    f8 = mybir.dt.float8e4
    AF = mybir.ActivationFunctionType
    OP = mybir.AluOpType
    DR = mybir.MatmulPerfMode.DoubleRow

    CS = int(sum(caps))
    offs = np.concatenate([[0], np.cumsum(caps)]).astype(int)
    CAP0 = int(max(caps))

    nc = bacc.Bacc("TRN2", target_bir_lowering=False, debug=False,
                   enable_asserts=False, num_devices=NCORE)

    XS = nc.dram_tensor("XS", [128, KC * TS], bf16, kind="ExternalInput").ap()
    WSG = nc.dram_tensor("WSG", [128, KC * SIH], bf16, kind="ExternalInput").ap()
    WSU = nc.dram_tensor("WSU", [128, KC * SIH], bf16, kind="ExternalInput").ap()
    WSD = nc.dram_tensor("WSD", [128, SICN * H], bf16, kind="ExternalInput").ap()
    XG8 = nc.dram_tensor("XG8", [128, KC2 * 2 * CS], f8, kind="ExternalInput").ap()
    WG8 = nc.dram_tensor("WG8", [EC, 128, KC2 * 2 * I], f8, kind="ExternalInput").ap()
    WU8 = nc.dram_tensor("WU8", [EC, 128, KC2 * 2 * I], f8, kind="ExternalInput").ap()
    WD8 = nc.dram_tensor("WD8", [EC, 128, 2 * 2 * H], f8, kind="ExternalInput").ap()
    Y = nc.dram_tensor("Y", [CS, H], bf16, kind="ExternalOutput").ap()
    YSH = nc.dram_tensor("YSH", [TS, H], bf16, kind="ExternalOutput").ap()

    with tile.TileContext(nc) as tc:
        with (
            tc.tile_pool(name="fixed", bufs=1) as fixed,
            tc.tile_pool(name="wpool", bufs=3) as wpool,
            tc.tile_pool(name="tmp", bufs=2) as tmp,
            tc.tile_pool(name="ypool", bufs=3) as ypool,
            tc.tile_pool(name="ps_s", bufs=4, space="PSUM") as ps_s,
            tc.tile_pool(name="ps_b", bufs=2, space="PSUM") as ps_b,
        ):
            # ---- input streams: XS on the scalar HWDGE ring (concurrent with
            # the sync ring), everything else on sync in consumption order.
            # WSG/WSU are loaded in interleaved per-sic chunks so the first
            # shared gate matmul can start ~1.5us in. ----
            xs_sb = fixed.tile([128, KC, TS], bf16, name="xs")
            nc.scalar.dma_start(xs_sb[:], XS.rearrange("p (kc t) -> p kc t", kc=KC))
            # sic-major host layout: each per-sic chunk is one contiguous DMA
            wsg_sb = fixed.tile([128, SICN, KC, 128], bf16, name="wsg")
            wsu_sb = fixed.tile([128, SICN, KC, 128], bf16, name="wsu")
            WSGr = WSG.rearrange("p (sc kc s) -> p sc kc s", sc=SICN, kc=KC)
            WSUr = WSU.rearrange("p (sc kc s) -> p sc kc s", sc=SICN, kc=KC)
            for sic in range(SICN):
                nc.sync.dma_start(wsg_sb[:, sic], WSGr[:, sic])
                nc.sync.dma_start(wsu_sb[:, sic], WSUr[:, sic])
            wsd_sb = fixed.tile([128, SICN, H], bf16, name="wsd")
            nc.sync.dma_start(wsd_sb[:], WSD.rearrange("p (sc h) -> p sc h", sc=SICN))
            xg_sb = fixed.tile([128, KC2, 2, CS], f8, name="xg")
            nc.sync.dma_start(xg_sb[:], XG8.rearrange("p (kk r c) -> p kk r c", kk=KC2, r=2))

            # expert weight stream (backpressured by wpool bufs=3)
            wg_sbs, wu_sbs, wd_sbs = [], [], []
            for e in range(EC):
                wg_sb = wpool.tile([128, KC2, 2, I], f8, tag="wg")
                nc.sync.dma_start(wg_sb[:], WG8[e].rearrange("p (kk r i) -> p kk r i", kk=KC2, r=2))
                wu_sb = wpool.tile([128, KC2, 2, I], f8, tag="wu")
                nc.sync.dma_start(wu_sb[:], WU8[e].rearrange("p (kk r i) -> p kk r i", kk=KC2, r=2))
                wd_sb = wpool.tile([128, 2, 2, H], f8, tag="wd")
                nc.sync.dma_start(wd_sb[:], WD8[e].rearrange("p (ii r h) -> p ii r h", ii=2, r=2))
                wg_sbs.append(wg_sb)
                wu_sbs.append(wu_sb)
                wd_sbs.append(wd_sb)

            # ---- PE warmup spin: flips the HAM clock gate to 2.4GHz while the
            # first input DMAs are still in flight ----
            wz = fixed.tile([128, 64], bf16, name="wz")
            nc.vector.memset(wz[:], 0.0)
            pwz = ps_s.tile([128, 512], f32, tag="mm_s")
            for _ in range(48):
                nc.tensor.matmul(pwz[0:64, 0:64], wz[:, 0:64], wz[:, 0:64],
                                 start=True, stop=True)

            # ---- shared MLP gate/up + silu*u (bf16) ----
            a_sh = fixed.tile([128, SICN, TS], bf16, name="a_sh")
            for sic in range(SICN):
                pg = ps_s.tile([128, 512], f32, tag="mm_s")
                pu = ps_s.tile([128, 512], f32, tag="mm_s")
                for kc in range(KC):
                    nc.tensor.matmul(pg[:, :TS], wsg_sb[:, sic, kc, :],
                                     xs_sb[:, kc, :], start=(kc == 0), stop=(kc == KC - 1))
                for kc in range(KC):
                    nc.tensor.matmul(pu[:, :TS], wsu_sb[:, sic, kc, :],
                                     xs_sb[:, kc, :], start=(kc == 0), stop=(kc == KC - 1))
                sg = tmp.tile([128, TS], f32, tag="ssilu")
                nc.scalar.activation(sg[:], pg[:, :TS], AF.Silu)
                nc.vector.tensor_tensor(a_sh[:, sic, :], sg[:], pu[:, :TS], op=OP.mult)

            def expert_gu(e):
                """fp8 DoubleRow gate/up + silu*u -> fp8 aT for expert slot e."""
                C = int(caps[e])
                off = int(offs[e])
                wg_sb, wu_sb = wg_sbs[e], wu_sbs[e]
                aT = tmp.tile([128, 2, 2, CAP0], f8, tag="aT")
                for ic in range(ICN):
                    pg = ps_s.tile([128, 512], f32, tag="mm_s")
                    pu = ps_s.tile([128, 512], f32, tag="mm_s")
                    for kk in range(KC2):
                        nc.tensor.matmul(pg[:, :C], wg_sb[:, kk, :, ic * 128:(ic + 1) * 128],
                                         xg_sb[:, kk, :, off:off + C],
                                         start=(kk == 0), stop=(kk == KC2 - 1),
                                         perf_mode=DR)
                    for kk in range(KC2):
                        nc.tensor.matmul(pu[:, :C], wu_sb[:, kk, :, ic * 128:(ic + 1) * 128],
                                         xg_sb[:, kk, :, off:off + C],
                                         start=(kk == 0), stop=(kk == KC2 - 1),
                                         perf_mode=DR)
                    sg = tmp.tile([128, CAP0], f32, tag="esilu")
                    nc.scalar.activation(sg[:, :C], pg[:, :C], AF.Silu, scale=DSC)
                    # aT = (silu * ASC/4096) * pu = ASC * a_true   (pu = 4096*u)
                    nc.vector.scalar_tensor_tensor(aT[:, ic // 2, ic % 2, 0:C],
                                                   sg[:, :C], ASC * DSC, pu[:, :C],
                                                   op0=OP.mult, op1=OP.mult)
                return aT

            def expert_down(e, aT):
                """fp8 DoubleRow down-proj for expert slot e (aT = ASC*a)."""
                C = int(caps[e])
                off = int(offs[e])
                wd_sb = wd_sbs[e]
                for cc in range((C + 127) // 128):
                    w = min(128, C - cc * 128)
                    py = ps_b.tile([128, H], f32, tag="mm_b")
                    for ii in range(2):
                        for nh in range(2):
                            nc.tensor.matmul(py[0:w, nh * 512:(nh + 1) * 512],
                                             aT[:, ii, :, cc * 128:cc * 128 + w],
                                             wd_sb[:, ii, :, nh * 512:(nh + 1) * 512],
                                             start=(ii == 0), stop=(ii == 1),
                                             perf_mode=DR)
                    yt = ypool.tile([128, H], bf16, tag="yt")
                    nc.scalar.activation(yt[0:w, 0:512], py[0:w, 0:512], AF.Copy, scale=DSC2)
                    nc.vector.tensor_scalar_mul(yt[0:w, 512:1024], py[0:w, 512:1024], DSC2)
                    nc.scalar.dma_start(Y[off + cc * 128: off + cc * 128 + w, :], yt[0:w, :])

            def shared_down():
                for tcc in range(TS // 128):
                    py = ps_b.tile([128, H], f32, tag="mm_b")
                    for sic in range(SICN):
                        for nh in range(2):
                            nc.tensor.matmul(py[:, nh * 512:(nh + 1) * 512],
                                             a_sh[:, sic, tcc * 128:(tcc + 1) * 128],
                                             wsd_sb[:, sic, nh * 512:(nh + 1) * 512],
                                             start=(sic == 0), stop=(sic == SICN - 1))
                    yt = ypool.tile([128, H], bf16, tag="yt")
                    nc.scalar.activation(yt[:, 0:512], py[:, 0:512], AF.Copy)
                    nc.vector.tensor_copy(yt[:, 512:1024], py[:, 512:1024])
                    nc.scalar.dma_start(YSH[tcc * 128:(tcc + 1) * 128, :], yt[:])

            # expert 0 gate/up first, then the shared down-proj fills the PE
            # while expert 0's silu/mult drains, then the expert pipeline
            aT0 = expert_gu(0)
            shared_down()
            expert_down(0, aT0)
            for e in range(1, EC):
                aT = expert_gu(e)
                expert_down(e, aT)

    nc.compile()
    return nc


def _route(inputs):
    """Replicate the reference router bit-exactly (same jax ops on CPU)."""
    x32 = np.ascontiguousarray(inputs["hidden_states"], dtype=np.float32)
    gw = np.ascontiguousarray(inputs["gate_w"], dtype=np.float32)
    gb = np.ascontiguousarray(inputs["gate_bias"], dtype=np.float32)
    try:
        import jax
        import jax.numpy as jnp
        cpu = jax.devices("cpu")[0]
        with jax.default_device(cpu):
            xs = jnp.asarray(x32)
            scores = jax.nn.sigmoid(xs @ jnp.asarray(gw).T)
            _, idx = jax.lax.top_k(scores + jnp.asarray(gb), K)
            w = jnp.take_along_axis(scores, idx, axis=1)
            w = w / jnp.sum(w, axis=1, keepdims=True)
            return np.asarray(idx), np.asarray(w, dtype=np.float64)
    except Exception:
        logits = x32 @ gw.T
        scores = (1.0 / (1.0 + np.exp(-logits, dtype=np.float32))).astype(np.float32)
        biased = scores + gb
        idx = np.argsort(-biased, axis=1, kind="stable")[:, :K]
        w = np.take_along_axis(scores, idx, axis=1).astype(np.float64)
        return idx, w / w.sum(axis=1, keepdims=True)


def _assign(idx):
    """Snake expert->(core,slot) assignment + 16-granular slot capacities."""
    counts = np.bincount(idx.ravel(), minlength=E)
    order = np.argsort(-counts, kind="stable")
    perm = np.zeros((NCORE, EC), dtype=np.int64)
    caps = []
    for s in range(EC):
        band = order[NCORE * s: NCORE * s + NCORE]
        perm[:, s] = band if s % 2 == 0 else band[::-1]
        caps.append(max(16, _rup(int(counts[band].max()), 16)))
    return perm, tuple(caps), counts


def _swz(a):
    """[H128*, N] -> [128, (H128*//128)*N] partition-swizzled, contiguous."""
    hh, n = a.shape
    return np.ascontiguousarray(
        a.reshape(hh // 128, 128, n).transpose(1, 0, 2).reshape(128, -1))


def _swz8(a):
    """[H, N] -> [128, (H//256)*2*N] fp8-DoubleRow layout: h = kk*256 + r*128 + p."""
    hh, n = a.shape
    return np.ascontiguousarray(
        a.reshape(hh // 256, 2, 128, n).transpose(2, 0, 1, 3).reshape(128, -1))


def _swz_sic(a):
    """[H, SIH] -> [128, SICN*KC*128] sic-major shared-weight layout."""
    return np.ascontiguousarray(
        a.reshape(KC, 128, SICN, 128).transpose(1, 2, 0, 3).reshape(128, -1))


def _prep(inputs, idx, wts, perm, caps, counts):
    import ml_dtypes
    bf = ml_dtypes.bfloat16
    f8 = ml_dtypes.float8_e4m3
    x = np.ascontiguousarray(inputs["hidden_states"], dtype=np.float32)
    w_gate = np.asarray(inputs["w_gate"], dtype=np.float32)
    w_up = np.asarray(inputs["w_up"], dtype=np.float32)
    w_down = np.asarray(inputs["w_down"], dtype=np.float32)
    ws_gate = np.asarray(inputs["ws_gate"], dtype=np.float32)
    ws_up = np.asarray(inputs["ws_up"], dtype=np.float32)
    ws_down = np.asarray(inputs["ws_down"], dtype=np.float32)

    xbf = x.astype(bf)
    xTbf = np.ascontiguousarray(xbf.T)        # [H, T] bf16
    x8 = (x * XSC).astype(f8)                 # [T, H] fp8, scaled
    CS = int(sum(caps))
    offs = np.concatenate([[0], np.cumsum(caps)]).astype(int)

    # per-expert routed token lists + combine weights (reference order)
    toks, wsel = [], []
    for e in range(E):
        mask = idx == e
        rows = np.nonzero(mask.any(axis=1))[0]
        toks.append(rows)
        wsel.append((wts * mask).sum(axis=1)[rows])

    in_maps, combine = [], []
    for c in range(NCORE):
        tb, sh = c // SGRP, c % SGRP
        g8 = np.zeros((CS, H), dtype=f8)
        rows_l, toks_l, wt_l = [], [], []
        for s in range(EC):
            e = int(perm[c, s])
            n = int(counts[e])
            g8[offs[s]:offs[s] + n] = x8[toks[e]]
            rows_l.append(offs[s] + np.arange(n))
            toks_l.append(toks[e])
            wt_l.append(wsel[e])
        in_maps.append({
            "XS": _swz(xTbf[:, tb * TS:(tb + 1) * TS]),
            "WSG": _swz_sic(ws_gate[:, sh * SIH:(sh + 1) * SIH].astype(bf)),
            "WSU": _swz_sic(ws_up[:, sh * SIH:(sh + 1) * SIH].astype(bf)),
            "WSD": _swz(ws_down[sh * SIH:(sh + 1) * SIH, :].astype(bf)),
            "XG8": _swz8(np.ascontiguousarray(g8.T)),
            "WG8": np.stack([_swz8((w_gate[int(perm[c, s])] * WSC).astype(f8))
                             for s in range(EC)]),
            "WU8": np.stack([_swz8((w_up[int(perm[c, s])] * WSC).astype(f8))
                             for s in range(EC)]),
            "WD8": np.stack([_swz8((w_down[int(perm[c, s])] * WSC).astype(f8))
                             for s in range(EC)]),
        })
        combine.append((np.concatenate(rows_l), np.concatenate(toks_l),
                        np.concatenate(wt_l)))
    return in_maps, combine


def _run(inputs, trace=False):
    from concourse import bass_utils
    idx, wts = _route(inputs)
    perm, caps, counts = _assign(idx)
    if caps not in _CACHE:
        _CACHE[caps] = _build(caps)
    nc = _CACHE[caps]
    in_maps, combine = _prep(inputs, idx, wts, perm, caps, counts)
    res = bass_utils.run_bass_kernel_spmd(nc, in_maps, core_ids=list(range(NCORE)),
                                          trace=trace)
    acc = np.zeros((T, H), dtype=np.float64)
    for c in range(NCORE):
        tb = c // SGRP
        acc[tb * TS:(tb + 1) * TS] += res.results[c]["YSH"].astype(np.float64)
        rows_c, toks_c, wt_c = combine[c]
        yc = res.results[c]["Y"][rows_c].astype(np.float64)
        np.add.at(acc, toks_c, yc * wt_c[:, None])
    return acc.astype(np.float32), res


def kernel(**inputs) -> np.ndarray:
    return _run(inputs, trace=False)[0]


# revision 18
# speedup vs baseline: 1.4424x; 1.0671x over previous
"""Ernie4 MoE (T=2048, H=1024, E=64 top-6, I=512 + shared SwiGLU, SI=1024) on 8 trn2 cores.

Host-routed expert parallelism. The router (fp32 gate logits, sigmoid, top-6,
renormalized combine weights) runs on host with the exact same jax-CPU ops as
the reference (bit-identical picks; min 6th/7th score gap on this data is
~2e-6, far above fp32 noise), and all routing-dependent data movement is done
host-side:
  * experts are assigned to (core, slot) by routed-count snake order; per-slot
    capacities (16-granular band maxima) are baked into the compiled program,
  * each core's routed token activations are pre-gathered into dense operands
    (zero padded), so the device never routes, gathers or scatters anything,
  * expert outputs come back dense [CS, H] bf16 in routed order; the host
    applies combine weights and scatter-adds in fp64.
The shared SwiGLU MLP is sharded 4-way over tokens x 2-way over SI
(core c: token block c>>1, SI half c&1), which minimizes its HBM traffic.

Precision split (rel err ~1.65e-2 vs the 2e-2 gate, numpy-emulated and
HW-verified to track emulation within 1%): the whole expert path runs
fp8(e4m3) with DoubleRow perf mode -- gate/up on (32*x, 128*w) with descale
1/4096 via the silu activation scale, down-proj on (16*a, 128*wd) with
descale 1/2048 at the PSUM->SBUF copy -- halving both expert PE cycles and
expert-weight HBM traffic. The shared MLP and all outputs stay bf16 (fp8
there pushes the error past the gate). fp32 PSUM accumulation everywhere.

Per core ~18.5MB in / 4.5MB out of HBM and ~67us of PE work: a short PE
warmup spin flips the HAM clock gate to 2.4GHz during the first DMA wait,
the shared MLP computes while expert weights stream on the sync HWDGE ring
(inputs) with outputs on the scalar ring, then 8 expert SwiGLU FFNs with
triple-buffered weight tiles. Shared down-proj is scheduled inside expert 0's
silu/mult latency gap.
"""

import numpy as np

T, H, E, K, I, SI = 2048, 1024, 64, 6, 512, 1024
NCORE = 8
EC = E // NCORE          # expert slots per core
KC = H // 128            # hidden-dim 128-chunks
KC2 = H // 256           # hidden-dim 256-chunks (fp8 DoubleRow)
ICN = I // 128           # expert-intermediate 128-chunks
TGRP, SGRP = 4, 2        # shared MLP sharding: token groups x SI groups
TS = T // TGRP           # shared tokens per core
SIH = SI // SGRP         # shared intermediate slice per core
SICN = SIH // 128
XSC, WSC = 32.0, 128.0   # fp8 scales for x and expert weights
ASC = 16.0               # fp8 scale for the expert intermediate activation
DSC = 1.0 / (XSC * WSC)  # gate/up PSUM descale
DSC2 = 1.0 / (ASC * WSC)  # down-proj PSUM descale

_CACHE = {}


def _rup(x, m):
    return (x + m - 1) // m * m


def _build(caps):
    """caps: per-slot FFN capacities (multiples of 16)."""
    import concourse.tile as tile
    from concourse import bacc, mybir

    f32 = mybir.dt.float32
    bf16 = mybir.dt.bfloat16
    f8 = mybir.dt.float8e4
    AF = mybir.ActivationFunctionType
    OP = mybir.AluOpType
    DR = mybir.MatmulPerfMode.DoubleRow

    CS = int(sum(caps))
    offs = np.concatenate([[0], np.cumsum(caps)]).astype(int)
    CAP0 = int(max(caps))

    nc = bacc.Bacc("TRN2", target_bir_lowering=False, debug=False,
                   enable_asserts=False, num_devices=NCORE)

    XS = nc.dram_tensor("XS", [128, KC * TS], bf16, kind="ExternalInput").ap()
    WSG = nc.dram_tensor("WSG", [128, KC * SIH], bf16, kind="ExternalInput").ap()
    WSU = nc.dram_tensor("WSU", [128, KC * SIH], bf16, kind="ExternalInput").ap()
    WSD = nc.dram_tensor("WSD", [128, SICN * H], bf16, kind="ExternalInput").ap()
    XG8 = nc.dram_tensor("XG8", [128, KC2 * 2 * CS], f8, kind="ExternalInput").ap()
    WG8 = nc.dram_tensor("WG8", [EC, 128, KC2 * 2 * I], f8, kind="ExternalInput").ap()
    WU8 = nc.dram_tensor("WU8", [EC, 128, KC2 * 2 * I], f8, kind="ExternalInput").ap()
    WD8 = nc.dram_tensor("WD8", [EC, 128, 2 * 2 * H], f8, kind="ExternalInput").ap()
    Y = nc.dram_tensor("Y", [CS, H], bf16, kind="ExternalOutput").ap()
    YSH = nc.dram_tensor("YSH", [TS, H], bf16, kind="ExternalOutput").ap()

    with tile.TileContext(nc) as tc:
        with (
            tc.tile_pool(name="fixed", bufs=1) as fixed,
            tc.tile_pool(name="wpool", bufs=5) as wpool,
            tc.tile_pool(name="tmp", bufs=2) as tmp,
            tc.tile_pool(name="ypool", bufs=4) as ypool,
            tc.tile_pool(name="ps_s", bufs=4, space="PSUM") as ps_s,
            tc.tile_pool(name="ps_b", bufs=2, space="PSUM") as ps_b,
        ):
            # ---- all inputs on the sync HWDGE ring, ordered by first use;
            # XS and WSG/WSU arrive in small chunks so the shared gate/up can
            # start consuming ~1.5us after the first bytes land. The scalar
            # ring carries only outputs. ----
            xs_sbs = []   # 4 token-chunk tiles of [128, KC, 128]
            XSr = XS.rearrange("p (tc kc t) -> p tc kc t", tc=4, kc=KC)
            wsg_sb = fixed.tile([128, SICN, KC, 128], bf16, name="wsg")
            wsu_sb = fixed.tile([128, SICN, KC, 128], bf16, name="wsu")
            WSGr = WSG.rearrange("p (sc kc s) -> p sc kc s", sc=SICN, kc=KC)
            WSUr = WSU.rearrange("p (sc kc s) -> p sc kc s", sc=SICN, kc=KC)
            nc.sync.dma_start(wsg_sb[:, 0], WSGr[:, 0])
            for tc4 in range(4):
                xs_t = fixed.tile([128, KC, 128], bf16, name=f"xs{tc4}")
                nc.sync.dma_start(xs_t[:], XSr[:, tc4])
                xs_sbs.append(xs_t)
                if tc4 == 0:
                    nc.sync.dma_start(wsu_sb[:, 0], WSUr[:, 0])
            for sic in range(1, SICN):
                nc.sync.dma_start(wsg_sb[:, sic], WSGr[:, sic])
                nc.sync.dma_start(wsu_sb[:, sic], WSUr[:, sic])
            xg_sb = fixed.tile([128, KC2, 2, CS], f8, name="xg")
            nc.sync.dma_start(xg_sb[:], XG8.rearrange("p (kk r c) -> p kk r c", kk=KC2, r=2))

            # expert weight stream (backpressured by wpool bufs), then the
            # shared down-proj weights (consumed last)
            wg_sbs, wu_sbs, wd_sbs = [], [], []
            for e in range(EC):
                wg_sb = wpool.tile([128, KC2, 2, I], f8, tag="wg")
                nc.sync.dma_start(wg_sb[:], WG8[e].rearrange("p (kk r i) -> p kk r i", kk=KC2, r=2))
                wu_sb = wpool.tile([128, KC2, 2, I], f8, tag="wu")
                nc.sync.dma_start(wu_sb[:], WU8[e].rearrange("p (kk r i) -> p kk r i", kk=KC2, r=2))
                wd_sb = wpool.tile([128, 2, 2, H], f8, tag="wd")
                nc.sync.dma_start(wd_sb[:], WD8[e].rearrange("p (ii r h) -> p ii r h", ii=2, r=2))
                wg_sbs.append(wg_sb)
                wu_sbs.append(wu_sb)
                wd_sbs.append(wd_sb)
            wsd_sb = fixed.tile([128, SICN, H], bf16, name="wsd")
            nc.sync.dma_start(wsd_sb[:], WSD.rearrange("p (sc h) -> p sc h", sc=SICN))

            # ---- PE warmup spin: flips the HAM clock gate toward 2.4GHz while
            # the first input DMAs are still in flight ----
            wz = fixed.tile([128, 64], bf16, name="wz")
            nc.vector.memset(wz[:], 0.0)
            pwz = ps_s.tile([128, 512], f32, tag="mm_s")
            for _ in range(40):
                nc.tensor.matmul(pwz[0:64, 0:64], wz[:, 0:64], wz[:, 0:64],
                                 start=True, stop=True)

            # ---- shared MLP gate/up + silu*u (bf16), token-chunk streaming ----
            a_sh = fixed.tile([128, SICN, TS], bf16, name="a_sh")
            for sic in range(SICN):
                pg = ps_s.tile([128, 512], f32, tag="mm_s")
                pu = ps_s.tile([128, 512], f32, tag="mm_s")
                for tc4 in range(4):
                    for kc in range(KC):
                        nc.tensor.matmul(pg[:, tc4 * 128:(tc4 + 1) * 128],
                                         wsg_sb[:, sic, kc, :], xs_sbs[tc4][:, kc, :],
                                         start=(kc == 0), stop=(kc == KC - 1))
                    for kc in range(KC):
                        nc.tensor.matmul(pu[:, tc4 * 128:(tc4 + 1) * 128],
                                         wsu_sb[:, sic, kc, :], xs_sbs[tc4][:, kc, :],
                                         start=(kc == 0), stop=(kc == KC - 1))
                sg = tmp.tile([128, TS], f32, tag="ssilu")
                nc.scalar.activation(sg[:], pg[:, :TS], AF.Silu)
                nc.vector.tensor_tensor(a_sh[:, sic, :], sg[:], pu[:, :TS], op=OP.mult)

            def expert_gu(e):
                """fp8 DoubleRow gate/up + silu*u -> fp8 aT for expert slot e."""
                C = int(caps[e])
                off = int(offs[e])
                wg_sb, wu_sb = wg_sbs[e], wu_sbs[e]
                aT = tmp.tile([128, 2, 2, CAP0], f8, tag="aT")
                for ic in range(ICN):
                    pg = ps_s.tile([128, 512], f32, tag="mm_s")
                    pu = ps_s.tile([128, 512], f32, tag="mm_s")
                    for kk in range(KC2):
                        nc.tensor.matmul(pg[:, :C], wg_sb[:, kk, :, ic * 128:(ic + 1) * 128],
                                         xg_sb[:, kk, :, off:off + C],
                                         start=(kk == 0), stop=(kk == KC2 - 1),
                                         perf_mode=DR)
                    for kk in range(KC2):
                        nc.tensor.matmul(pu[:, :C], wu_sb[:, kk, :, ic * 128:(ic + 1) * 128],
                                         xg_sb[:, kk, :, off:off + C],
                                         start=(kk == 0), stop=(kk == KC2 - 1),
                                         perf_mode=DR)
                    sg = tmp.tile([128, CAP0], f32, tag="esilu")
                    nc.scalar.activation(sg[:, :C], pg[:, :C], AF.Silu, scale=DSC)
                    # aT = (silu * ASC/4096) * pu = ASC * a_true   (pu = 4096*u)
                    nc.vector.scalar_tensor_tensor(aT[:, ic // 2, ic % 2, 0:C],
                                                   sg[:, :C], ASC * DSC, pu[:, :C],
                                                   op0=OP.mult, op1=OP.mult)
                return aT

            def expert_down(e, aT):
                """fp8 DoubleRow down-proj for expert slot e (aT = ASC*a)."""
                C = int(caps[e])
                off = int(offs[e])
                wd_sb = wd_sbs[e]
                for cc in range((C + 127) // 128):
                    w = min(128, C - cc * 128)
                    py = ps_b.tile([128, H], f32, tag="mm_b")
                    for ii in range(2):
                        for nh in range(2):
                            nc.tensor.matmul(py[0:w, nh * 512:(nh + 1) * 512],
                                             aT[:, ii, :, cc * 128:cc * 128 + w],
                                             wd_sb[:, ii, :, nh * 512:(nh + 1) * 512],
                                             start=(ii == 0), stop=(ii == 1),
                                             perf_mode=DR)
                    yt = ypool.tile([128, H], bf16, tag="yt")
                    nc.scalar.activation(yt[0:w, 0:384], py[0:w, 0:384], AF.Copy, scale=DSC2)
                    nc.vector.tensor_scalar_mul(yt[0:w, 384:1024], py[0:w, 384:1024], DSC2)
                    nc.scalar.dma_start(Y[off + cc * 128: off + cc * 128 + w, :], yt[0:w, :])

            def shared_down():
                for tcc in range(TS // 128):
                    py = ps_b.tile([128, H], f32, tag="mm_b")
                    for sic in range(SICN):
                        for nh in range(2):
                            nc.tensor.matmul(py[:, nh * 512:(nh + 1) * 512],
                                             a_sh[:, sic, tcc * 128:(tcc + 1) * 128],
                                             wsd_sb[:, sic, nh * 512:(nh + 1) * 512],
                                             start=(sic == 0), stop=(sic == SICN - 1))
                    yt = ypool.tile([128, H], bf16, tag="yt")
                    nc.scalar.activation(yt[:, 0:384], py[:, 0:384], AF.Copy)
                    nc.vector.tensor_copy(yt[:, 384:1024], py[:, 384:1024])
                    nc.scalar.dma_start(YSH[tcc * 128:(tcc + 1) * 128, :], yt[:])

            # software-pipelined expert emission -- gu(e+1) is queued before
            # down(e) so the PE never waits on e's silu/mult latency; the
            # shared down-proj (no DMA dependency) fills the tail.
            aTs = [expert_gu(0)]
            for e in range(1, EC):
                aTs.append(expert_gu(e))
                expert_down(e - 1, aTs[e - 1])
                aTs[e - 1] = None
            expert_down(EC - 1, aTs[EC - 1])
            shared_down()

    nc.compile()
    return nc


def _route(inputs):
    """Replicate the reference router bit-exactly (same jax ops on CPU)."""
    x32 = np.ascontiguousarray(inputs["hidden_states"], dtype=np.float32)
    gw = np.ascontiguousarray(inputs["gate_w"], dtype=np.float32)
    gb = np.ascontiguousarray(inputs["gate_bias"], dtype=np.float32)
    try:
        import jax
        import jax.numpy as jnp
        cpu = jax.devices("cpu")[0]
        with jax.default_device(cpu):
            xs = jnp.asarray(x32)
            scores = jax.nn.sigmoid(xs @ jnp.asarray(gw).T)
            _, idx = jax.lax.top_k(scores + jnp.asarray(gb), K)
            w = jnp.take_along_axis(scores, idx, axis=1)
            w = w / jnp.sum(w, axis=1, keepdims=True)
            return np.asarray(idx), np.asarray(w, dtype=np.float64)
    except Exception:
        logits = x32 @ gw.T
        scores = (1.0 / (1.0 + np.exp(-logits, dtype=np.float32))).astype(np.float32)
        biased = scores + gb
        idx = np.argsort(-biased, axis=1, kind="stable")[:, :K]
        w = np.take_along_axis(scores, idx, axis=1).astype(np.float64)
        return idx, w / w.sum(axis=1, keepdims=True)


def _assign(idx):
    """Snake expert->(core,slot) assignment + 16-granular slot capacities."""
    counts = np.bincount(idx.ravel(), minlength=E)
    order = np.argsort(-counts, kind="stable")
    perm = np.zeros((NCORE, EC), dtype=np.int64)
    caps = []
    for s in range(EC):
        band = order[NCORE * s: NCORE * s + NCORE]
        perm[:, s] = band if s % 2 == 0 else band[::-1]
        caps.append(max(16, _rup(int(counts[band].max()), 16)))
    return perm, tuple(caps), counts


def _swz(a):
    """[H128*, N] -> [128, (H128*//128)*N] partition-swizzled, contiguous."""
    hh, n = a.shape
    return np.ascontiguousarray(
        a.reshape(hh // 128, 128, n).transpose(1, 0, 2).reshape(128, -1))


def _swz8(a):
    """[H, N] -> [128, (H//256)*2*N] fp8-DoubleRow layout: h = kk*256 + r*128 + p."""
    hh, n = a.shape
    return np.ascontiguousarray(
        a.reshape(hh // 256, 2, 128, n).transpose(2, 0, 1, 3).reshape(128, -1))


def _swz_sic(a):
    """[H, SIH] -> [128, SICN*KC*128] sic-major shared-weight layout."""
    return np.ascontiguousarray(
        a.reshape(KC, 128, SICN, 128).transpose(1, 2, 0, 3).reshape(128, -1))


def _prep(inputs, idx, wts, perm, caps, counts):
    import ml_dtypes
    bf = ml_dtypes.bfloat16
    f8 = ml_dtypes.float8_e4m3
    x = np.ascontiguousarray(inputs["hidden_states"], dtype=np.float32)
    w_gate = np.asarray(inputs["w_gate"], dtype=np.float32)
    w_up = np.asarray(inputs["w_up"], dtype=np.float32)
    w_down = np.asarray(inputs["w_down"], dtype=np.float32)
    ws_gate = np.asarray(inputs["ws_gate"], dtype=np.float32)
    ws_up = np.asarray(inputs["ws_up"], dtype=np.float32)
    ws_down = np.asarray(inputs["ws_down"], dtype=np.float32)

    xbf = x.astype(bf)
    xTbf = np.ascontiguousarray(xbf.T)        # [H, T] bf16
    x8 = (x * XSC).astype(f8)                 # [T, H] fp8, scaled
    CS = int(sum(caps))
    offs = np.concatenate([[0], np.cumsum(caps)]).astype(int)

    # per-expert routed token lists + combine weights (reference order)
    toks, wsel = [], []
    for e in range(E):
        mask = idx == e
        rows = np.nonzero(mask.any(axis=1))[0]
        toks.append(rows)
        wsel.append((wts * mask).sum(axis=1)[rows])

    in_maps, combine = [], []
    for c in range(NCORE):
        tb, sh = c // SGRP, c % SGRP
        g8 = np.zeros((CS, H), dtype=f8)
        rows_l, toks_l, wt_l = [], [], []
        for s in range(EC):
            e = int(perm[c, s])
            n = int(counts[e])
            g8[offs[s]:offs[s] + n] = x8[toks[e]]
            rows_l.append(offs[s] + np.arange(n))
            toks_l.append(toks[e])
            wt_l.append(wsel[e])
        xs_slice = xTbf[:, tb * TS:(tb + 1) * TS]     # [H, TS]
        xs_tc = np.ascontiguousarray(                 # [128, tc, kc, 128] token-chunked
            xs_slice.reshape(KC, 128, 4, 128).transpose(1, 2, 0, 3).reshape(128, -1))
        in_maps.append({
            "XS": xs_tc,
            "WSG": _swz_sic(ws_gate[:, sh * SIH:(sh + 1) * SIH].astype(bf)),
            "WSU": _swz_sic(ws_up[:, sh * SIH:(sh + 1) * SIH].astype(bf)),
            "WSD": _swz(ws_down[sh * SIH:(sh + 1) * SIH, :].astype(bf)),
            "XG8": _swz8(np.ascontiguousarray(g8.T)),
            "WG8": np.stack([_swz8((w_gate[int(perm[c, s])] * WSC).astype(f8))
                             for s in range(EC)]),
            "WU8": np.stack([_swz8((w_up[int(perm[c, s])] * WSC).astype(f8))
                             for s in range(EC)]),
            "WD8": np.stack([_swz8((w_down[int(perm[c, s])] * WSC).astype(f8))
                             for s in range(EC)]),
        })
        combine.append((np.concatenate(rows_l), np.concatenate(toks_l),
                        np.concatenate(wt_l)))
    return in_maps, combine


def _run(inputs, trace=False):
    from concourse import bass_utils
    idx, wts = _route(inputs)
    perm, caps, counts = _assign(idx)
    if caps not in _CACHE:
        _CACHE[caps] = _build(caps)
    nc = _CACHE[caps]
    in_maps, combine = _prep(inputs, idx, wts, perm, caps, counts)
    res = bass_utils.run_bass_kernel_spmd(nc, in_maps, core_ids=list(range(NCORE)),
                                          trace=trace)
    acc = np.zeros((T, H), dtype=np.float64)
    for c in range(NCORE):
        tb = c // SGRP
        acc[tb * TS:(tb + 1) * TS] += res.results[c]["YSH"].astype(np.float64)
        rows_c, toks_c, wt_c = combine[c]
        yc = res.results[c]["Y"][rows_c].astype(np.float64)
        np.add.at(acc, toks_c, yc * wt_c[:, None])
    return acc.astype(np.float32), res


def kernel(**inputs) -> np.ndarray:
    return _run(inputs, trace=False)[0]
